# revision 10
# baseline (speedup 1.0000x reference)
"""AZConv2d fused anisotropic conv kernel for Trainium2 (Bass/Tile).

Math (per pixel l, rule r):
  gate = gate_w@x + gate_b; mu = softmax_r(gate)
  v = value_w@x;  geom = geom_w@x + geom_b -> theta, raw_base, raw_hyper (4 each)
  base = softplus(rb)+1e-4; hyper = softplus(rh)+0.1
  iu2 = 1/(base*e^h)^2; is2 = 1/(base*e^-h)^2
  kern(dy,dx) = exp(-(a*dx^2 + b*dy^2 + c2*dx*dy)),
     a = ct^2*iu2 + st^2*is2, b = st^2*iu2+ct^2*is2, c2 = 2*ct*st*(iu2-is2)
  w = mu*kern / (sum_{r,s} mu*kern + 1e-6)
  agg[r,c] = sum_s w[r,s] * v[c, l+delta_s];  out = pw_w @ agg + pw_b

Since dx,dy in {-1,0,1}, kern takes 5 distinct values {1, A, B, AB*e^-c2, AB*e^c2}
(A=e^-a, B=e^-b), so with the mirrored-shift pair sums
  V0=v, V1=v(l-1)+v(l+1), V2=v(l-W)+v(l+W),
  S=V3+V4, T=V3-V4  (V3/V4 = main/anti diagonal pairs)
agg_r = G0*V0 + G1*V1 + G2*V2 + G3*S + G4*T with per-pixel scalars
  G0=em', G1=em'A, G2=em'B, G3=em'AB*cosh(c2), G4=-em'AB*sinh(c2),
  em' = e^{gate-max}/(sum_r e^{gate-max}*Sk_r + 1e-6*sum_r e^{gate-max}),
  Sk = 1+2A+2B+4AB*cosh(c2).
S/T come from horizontal pair sum/diff fields propagated via value conv:
  u = Wv@(x(l-1)+x(l+1)), wd = Wv@(x(l-1)-x(l+1));
  V1 = u, S = u(l-W)+u(l+W), T = wd(l-W)-wd(l+W).

Sharding: data-parallel over batch, 1 image per NeuronCore (8 cores).
Layout: pixel-major bands of 8 image rows; partitions = 128 pixels of a row.
"""

import math
import sys

for p in ("/opt/trn_rl_repo",):
    if p not in sys.path:
        sys.path.insert(0, p)

import numpy as np

import concourse.bass as bass
import concourse.mybir as mybir
import concourse.tile as tile
from concourse import bacc
from concourse.bass import ds
from concourse.masks import make_identity

F32 = mybir.dt.float32
AF = mybir.ActivationFunctionType
ALU = mybir.AluOpType

B = 8
C = 64
H = 128
W = 128
L = H * W
R = 4
BAND = 8
NBANDS = H // BAND  # 16
PAIR_ROWS = 2 * BAND  # geometry granularity (16 rows)
EPS = 1e-4
MIN_HYP = 0.1
LN2 = math.log(2.0)


def _production(nc, pools, x2, rhs0, Bi, gg):
    """Produce pixel-major fields for band Bi (rows y0..y0+7) incl. 1-row halo.

    Returns (v_pm, u_pm, w_pm, v2, vs, vt): [128, rows, 64] tiles.
    v/u/w have 10 rows (j=0..9 -> image rows y0-1..y0+8, zero at image edge);
    v2/vs/vt have 8 rows (the band itself).
    Also copies this band's gate/geom (16 fields) into gg rows.
    """
    pband, pps, pact = pools["band"], pools["ps"], pools["act"]
    y0 = Bi * BAND
    jlo = 1 if Bi == 0 else 0
    jhi = 9 if Bi == NBANDS - 1 else 10
    half = 0 if Bi < NBANDS // 2 else 1
    p0 = 64 * half
    rowbase = 0 if half == 0 else 63  # x2 half-1 rows 0..64, half-2 rows 63..127

    def xrow(y):
        return x2[p0 : p0 + 64, ds((y - rowbase) * W, W)]

    n = jhi - jlo
    # c-major slice of x covering rows y0-1+jlo .. y0-1+jhi-1
    x2v = x2[p0 : p0 + 64, ds((y0 - 1 + jlo - rowbase) * W, n * W)].rearrange(
        "p (r w) -> p r w", w=W
    )

    # horizontal pair sum / diff of x (c-major), with zero-pad column fixes
    # (allocated full-height; only partitions p0..p0+64 are used, matching x2's half)
    x1f = pband.tile([128, 10, W], F32, tag="x1")
    xdf = pband.tile([128, 10, W], F32, tag="xd")
    x1 = x1f[p0 : p0 + 64]
    xd = xdf[p0 : p0 + 64]
    nc.vector.tensor_tensor(
        out=x1[:, jlo:jhi, 1 : W - 1],
        in0=x2v[:, :, 0 : W - 2],
        in1=x2v[:, :, 2:W],
        op=ALU.add,
    )
    nc.vector.tensor_copy(x1[:, jlo:jhi, 0:1], x2v[:, :, 1:2])
    nc.vector.tensor_copy(x1[:, jlo:jhi, W - 1 : W], x2v[:, :, W - 2 : W - 1])
    nc.vector.tensor_tensor(
        out=xd[:, jlo:jhi, 1 : W - 1],
        in0=x2v[:, :, 0 : W - 2],
        in1=x2v[:, :, 2:W],
        op=ALU.subtract,
    )
    nc.vector.tensor_scalar(
        out=xd[:, jlo:jhi, 0:1],
        in0=x2v[:, :, 1:2],
        scalar1=-1.0,
        scalar2=None,
        op0=ALU.mult,
    )
    nc.vector.tensor_copy(xd[:, jlo:jhi, W - 1 : W], x2v[:, :, W - 2 : W - 1])

    v_pm = pband.tile([128, 10, 64], F32, tag="v")
    u_pm = pband.tile([128, 10, 64], F32, tag="u")
    w_pm = pband.tile([128, 10, 64], F32, tag="w")

    # value/gate/geom projections, pixel-major: out[pix, f] via lhsT = x row
    for g in range(2):  # groups of 5 rows for the 80-col projection
        ga, gb_ = max(g * 5, jlo), min(g * 5 + 5, jhi)
        if ga >= gb_:
            continue
        ps_v = pps.tile([128, 5, 80], F32, tag="psv")
        for j in range(ga, gb_):
            nc.tensor.matmul(
                out=ps_v[:, j - g * 5, :],
                lhsT=xrow(y0 - 1 + j),
                rhs=rhs0[p0 : p0 + 64, :],
                start=True,
                stop=True,
            )
        nc.scalar.activation(
            out=v_pm[:, ga:gb_, :], in_=ps_v[:, ga - g * 5 : gb_ - g * 5, 0:64],
            func=AF.Copy,
        )
        # gate/geom for the band's own rows (j=1..8) into the pair tile gg
        ia, ib = max(ga, 1), min(gb_, 9)
        if ia < ib:
            ggrow = (y0 % PAIR_ROWS) + ia - 1
            nc.scalar.activation(
                out=gg[:, ggrow : ggrow + (ib - ia), :],
                in_=ps_v[:, ia - g * 5 : ib - g * 5, 64:80],
                func=AF.Copy,
            )

    for g in range(3):  # groups of 4 rows for u/w projections
        ga, gb_ = max(g * 4, jlo), min(g * 4 + 4, jhi)
        if ga >= gb_:
            continue
        ps_uw = pps.tile([128, 4, 128], F32, tag="psuw")
        for j in range(ga, gb_):
            nc.tensor.matmul(
                out=ps_uw[:, j - g * 4, 0:64],
                lhsT=x1[:, j, :],
                rhs=rhs0[p0 : p0 + 64, 0:64],
                start=True,
                stop=True,
            )
            nc.tensor.matmul(
                out=ps_uw[:, j - g * 4, 64:128],
                lhsT=xd[:, j, :],
                rhs=rhs0[p0 : p0 + 64, 0:64],
                start=True,
                stop=True,
            )
        nc.scalar.activation(
            out=u_pm[:, ga:gb_, :], in_=ps_uw[:, ga - g * 4 : gb_ - g * 4, 0:64],
            func=AF.Copy,
        )
        nc.scalar.activation(
            out=w_pm[:, ga:gb_, :], in_=ps_uw[:, ga - g * 4 : gb_ - g * 4, 64:128],
            func=AF.Copy,
        )

    # zero-pad rows outside the image
    if jlo == 1:
        for t in (v_pm, u_pm, w_pm):
            nc.gpsimd.memset(t[:, 0:1, :], 0.0)
    if jhi == 9:
        for t in (v_pm, u_pm, w_pm):
            nc.gpsimd.memset(t[:, 9:10, :], 0.0)

    # vertical pair fields for the band
    v2 = pband.tile([128, BAND, 64], F32, tag="v2")
    vs = pband.tile([128, BAND, 64], F32, tag="vs")
    vt = pband.tile([128, BAND, 64], F32, tag="vt")
    nc.vector.tensor_tensor(out=v2, in0=v_pm[:, 0:8, :], in1=v_pm[:, 2:10, :], op=ALU.add)
    nc.vector.tensor_tensor(out=vs, in0=u_pm[:, 0:8, :], in1=u_pm[:, 2:10, :], op=ALU.add)
    nc.vector.tensor_tensor(
        out=vt, in0=w_pm[:, 0:8, :], in1=w_pm[:, 2:10, :], op=ALU.subtract
    )
    return v_pm, u_pm, w_pm, v2, vs, vt


def _geometry(nc, pools, gg, gbias, mbias, cb):
    """Per-pixel aggregation coefficients G0..G4 [128, PAIR_ROWS, 4] from gg."""
    pg = pools["geo"]
    Y = PAIR_ROWS

    def gt(tag):
        return pg.tile([128, Y, R], F32, tag=tag, name=tag)

    gate = gg[:, :, 0:4]
    theta = gg[:, :, 4:8]

    # biases (broadcast along rows)
    nc.vector.tensor_tensor(
        out=gate, in0=gate, in1=gbias[:, None, :].broadcast_to([128, Y, 4]), op=ALU.add
    )
    nc.vector.tensor_tensor(
        out=gg[:, :, 4:16],
        in0=gg[:, :, 4:16],
        in1=mbias[:, None, :].broadcast_to([128, Y, 12]),
        op=ALU.add,
    )

    mx = pg.tile([128, Y, 1], F32, tag="mx")
    nc.vector.tensor_reduce(out=mx, in_=gate, axis=mybir.AxisListType.X, op=ALU.max)
    gsh = gt("gsh")
    nc.vector.tensor_tensor(
        out=gsh, in0=gate, in1=mx.broadcast_to([128, Y, 4]), op=ALU.subtract
    )
    em = gt("em")
    nc.scalar.activation(out=em, in_=gsh, func=AF.Exp)

    e8 = pg.tile([128, Y, 8], F32, tag="e8")
    nc.scalar.activation(out=e8, in_=gg[:, :, 8:16], func=AF.Exp)
    sp8 = pg.tile([128, Y, 8], F32, tag="sp8")
    nc.scalar.activation(out=sp8, in_=e8, func=AF.Ln, bias=1.0)
    spb, sph = sp8[:, :, 0:4], sp8[:, :, 4:8]
    lb = gt("lb")
    nc.scalar.activation(out=lb, in_=spb, func=AF.Ln, bias=cb["eps"])
    ct, st = gt("ct"), gt("st")
    sh = gt("sh")
    nc.scalar.activation(out=sh, in_=theta, func=AF.Sin, scale=0.5)
    sh2 = gt("sh2")
    nc.scalar.activation(out=sh2, in_=sh, func=AF.Square)
    nc.vector.tensor_scalar(
        out=ct, in0=sh2, scalar1=-2.0, scalar2=1.0, op0=ALU.mult, op1=ALU.add
    )
    nc.scalar.activation(out=st, in_=theta, func=AF.Sin)

    tpl, tmi = gt("tpl"), gt("tmi")
    nc.vector.tensor_tensor(out=tpl, in0=sph, in1=lb, op=ALU.add)
    nc.vector.tensor_tensor(out=tmi, in0=sph, in1=lb, op=ALU.subtract)
    iu2, is2 = gt("iu2"), gt("is2")
    # 1/sig_u^2 = exp(-2(sph+0.1+lb)); 1/sig_s^2 = exp(2(sph+0.1-lb))
    nc.scalar.activation(out=iu2, in_=tpl, func=AF.Exp, scale=-2.0, bias=cb["mh2n"])
    nc.scalar.activation(out=is2, in_=tmi, func=AF.Exp, scale=2.0, bias=cb["mh2p"])

    ct2, st2 = gt("ct2"), gt("st2")
    nc.scalar.activation(out=ct2, in_=ct, func=AF.Square)
    nc.scalar.activation(out=st2, in_=st, func=AF.Square)

    t1, t2, av = gt("t1"), gt("t2"), gt("av")
    nc.vector.tensor_tensor(out=t1, in0=ct2, in1=iu2, op=ALU.mult)
    nc.vector.tensor_tensor(out=t2, in0=st2, in1=is2, op=ALU.mult)
    nc.vector.tensor_tensor(out=av, in0=t1, in1=t2, op=ALU.add)
    ssum, bv = gt("ssum"), gt("bv")
    nc.vector.tensor_tensor(out=ssum, in0=iu2, in1=is2, op=ALU.add)
    nc.vector.tensor_tensor(out=bv, in0=ssum, in1=av, op=ALU.subtract)

    cs, dio, c2h = gt("cs"), gt("dio"), gt("c2h")
    nc.vector.tensor_tensor(out=cs, in0=ct, in1=st, op=ALU.mult)
    nc.vector.tensor_tensor(out=dio, in0=iu2, in1=is2, op=ALU.subtract)
    nc.vector.tensor_tensor(out=c2h, in0=cs, in1=dio, op=ALU.mult)

    Aa, Bb = gt("Aa"), gt("Bb")
    nc.scalar.activation(out=Aa, in_=av, func=AF.Exp, scale=-1.0)
    nc.scalar.activation(out=Bb, in_=bv, func=AF.Exp, scale=-1.0)
    # Corner kernels, overflow-safe: a+b = ssum >= |c2|, so compute
    # P/2 = exp(-(a+b+c2))/2 and Q/2 = exp(-(a+b-c2))/2 directly.
    sc2, sc2m = gt("sc2"), gt("sc2m")
    nc.vector.scalar_tensor_tensor(
        out=sc2, in0=c2h, scalar=2.0, in1=ssum, op0=ALU.mult, op1=ALU.add
    )
    nc.vector.scalar_tensor_tensor(
        out=sc2m, in0=c2h, scalar=-2.0, in1=ssum, op0=ALU.mult, op1=ALU.add
    )
    ph, qh = gt("ph"), gt("qh")
    nc.scalar.activation(out=ph, in_=sc2, func=AF.Exp, scale=-1.0, bias=cb["mln2"])
    nc.scalar.activation(out=qh, in_=sc2m, func=AF.Exp, scale=-1.0, bias=cb["mln2"])
    ppqh, pmqh = gt("ppqh"), gt("pmqh")
    nc.vector.tensor_tensor(out=ppqh, in0=ph, in1=qh, op=ALU.add)
    nc.vector.tensor_tensor(out=pmqh, in0=ph, in1=qh, op=ALU.subtract)

    # Sk = 1 + 2(A+B) + 4*(P+Q)/2
    apb, w1, sk = gt("apb"), gt("w1"), gt("sk")
    nc.vector.tensor_tensor(out=apb, in0=Aa, in1=Bb, op=ALU.add)
    nc.vector.scalar_tensor_tensor(
        out=w1, in0=ppqh, scalar=2.0, in1=apb, op0=ALU.mult, op1=ALU.add
    )
    nc.vector.tensor_scalar(
        out=sk, in0=w1, scalar1=2.0, scalar2=1.0, op0=ALU.mult, op1=ALU.add
    )

    ws = gt("ws")
    nc.vector.tensor_tensor(out=ws, in0=em, in1=sk, op=ALU.mult)
    Dp = pg.tile([128, Y, 1], F32, tag="Dp")
    Smu = pg.tile([128, Y, 1], F32, tag="Smu")
    nc.vector.tensor_reduce(out=Dp, in_=ws, axis=mybir.AxisListType.X, op=ALU.add)
    nc.vector.tensor_reduce(out=Smu, in_=em, axis=mybir.AxisListType.X, op=ALU.add)
    D2 = pg.tile([128, Y, 1], F32, tag="D2")
    nc.vector.scalar_tensor_tensor(
        out=D2, in0=Smu, scalar=1e-6, in1=Dp, op0=ALU.mult, op1=ALU.add
    )
    invD = pg.tile([128, Y, 1], F32, tag="invD")
    nc.vector.reciprocal(invD, D2)

    em2 = gt("em2")
    nc.vector.tensor_tensor(
        out=em2, in0=em, in1=invD.broadcast_to([128, Y, 4]), op=ALU.mult
    )
    G1, G2, G3, G4 = gt("G1"), gt("G2"), gt("G3"), gt("G4")
    nc.vector.tensor_tensor(out=G1, in0=em2, in1=Aa, op=ALU.mult)
    nc.vector.tensor_tensor(out=G2, in0=em2, in1=Bb, op=ALU.mult)
    nc.vector.tensor_tensor(out=G3, in0=em2, in1=ppqh, op=ALU.mult)
    nc.vector.tensor_tensor(out=G4, in0=em2, in1=pmqh, op=ALU.mult)
    return em2, G1, G2, G3, G4


def _mac_and_out(nc, pools, Bi, bt, G, goff, ident, pwt, pwb, out_d):
    """Weighted aggregation for band Bi, transpose to c-major, pointwise conv."""
    pband, pma, ptmp, ppsT, ppso, pout = (
        pools["band"],
        pools["mac"],
        pools["tmp"],
        pools["psT"],
        pools["pso"],
        pools["out"],
    )
    v_pm, u_pm, w_pm, v2, vs, vt = bt
    G0, G1, G2, G3, G4 = G
    y0 = Bi * BAND
    sh = [128, BAND, R, 64]

    def bg(t):  # coeff [128, 8, 4] -> bcast over channels
        return t[:, goff : goff + BAND, :, None].broadcast_to(sh)

    def bv(t):  # field [128, 8, 64] -> bcast over rules
        return t[:, :, None, :].broadcast_to(sh)

    agg = pma.tile(sh, F32, tag="agg")
    tA = ptmp.tile(sh, F32, tag="tA")
    tB = ptmp.tile(sh, F32, tag="tB")
    nc.vector.tensor_tensor(out=tA, in0=bv(v_pm[:, 1:9, :]), in1=bg(G0), op=ALU.mult)
    nc.vector.tensor_tensor(out=tB, in0=bv(u_pm[:, 1:9, :]), in1=bg(G1), op=ALU.mult)
    nc.vector.tensor_tensor(out=agg, in0=tA, in1=tB, op=ALU.add)
    nc.vector.tensor_tensor(out=tA, in0=bv(v2), in1=bg(G2), op=ALU.mult)
    nc.vector.tensor_tensor(out=agg, in0=agg, in1=tA, op=ALU.add)
    nc.vector.tensor_tensor(out=tB, in0=bv(vs), in1=bg(G3), op=ALU.mult)
    nc.vector.tensor_tensor(out=agg, in0=agg, in1=tB, op=ALU.add)
    nc.vector.tensor_tensor(out=tA, in0=bv(vt), in1=bg(G4), op=ALU.mult)
    nc.vector.tensor_tensor(out=agg, in0=agg, in1=tA, op=ALU.add)

    # transpose [pixel, (r c)] -> [(r c), pixel] via PE, half (128 rc) at a time
    aggT = pma.tile([128, 2, BAND, 128], F32, tag="aggT")
    for hg in range(2):
        for grp in range(2):
            psT = ppsT.tile([128, 4, 128], F32, tag="psT")
            for yy in range(4):
                yr = grp * 4 + yy
                nc.tensor.transpose(
                    out=psT[:, yy, :],
                    in_=agg[:, yr, 2 * hg : 2 * hg + 2, :].rearrange(
                        "p a b -> p (a b)"
                    ),
                    identity=ident,
                )
            nc.scalar.activation(
                out=aggT[:, hg, grp * 4 : grp * 4 + 4, :], in_=psT, func=AF.Copy
            )

    # out = pw @ agg + pw_b (c-major), 512 pixels per psum chunk
    for ch in range(2):
        ps_o = ppso.tile([64, 512], F32, tag="pso")
        for hg in range(2):
            nc.tensor.matmul(
                out=ps_o,
                lhsT=pwt[:, hg, :],
                rhs=aggT[:, hg, ch * 4 : ch * 4 + 4, :].rearrange("p a b -> p (a b)"),
                start=(hg == 0),
                stop=(hg == 1),
            )
        osb = pout.tile([64, 512], F32, tag="osb")
        nc.scalar.activation(out=osb, in_=ps_o, func=AF.Identity, bias=pwb, scale=1.0)
        nc.sync.dma_start(out=out_d[:, ds(y0 * W + ch * 512, 512)], in_=osb)


def build_nc():
    nc = bacc.Bacc("TRN2")
    x_d = nc.dram_tensor("x", [C, L], F32, kind="ExternalInput").ap()
    rhs0_d = nc.dram_tensor("rhs0", [C, 80], F32, kind="ExternalInput").ap()
    pwt_d = nc.dram_tensor("pw_t2", [128, 2, 64], F32, kind="ExternalInput").ap()
    gb_d = nc.dram_tensor("gate_b", [R], F32, kind="ExternalInput").ap()
    mb_d = nc.dram_tensor("geom_b", [12], F32, kind="ExternalInput").ap()
    pb_d = nc.dram_tensor("pw_b", [C], F32, kind="ExternalInput").ap()
    out_d = nc.dram_tensor("out", [C, L], F32, kind="ExternalOutput").ap()

    with tile.TileContext(nc) as tc:
        import contextlib

        with contextlib.ExitStack() as ctx:
            persist = ctx.enter_context(tc.tile_pool(name="persist", bufs=1))
            pools = {
                "band": ctx.enter_context(tc.tile_pool(name="band", bufs=3)),
                "mac": ctx.enter_context(tc.tile_pool(name="mac", bufs=2)),
                "tmp": ctx.enter_context(tc.tile_pool(name="tmp", bufs=1)),
                "geo": ctx.enter_context(tc.tile_pool(name="geo", bufs=2)),
                "gg": ctx.enter_context(tc.tile_pool(name="gg", bufs=2)),
                "act": None,
                "out": ctx.enter_context(tc.tile_pool(name="out", bufs=2)),
                "ps": ctx.enter_context(tc.tile_pool(name="ps", bufs=2, space="PSUM")),
                "psT": ctx.enter_context(
                    tc.tile_pool(name="psT", bufs=2, space="PSUM")
                ),
                "pso": ctx.enter_context(
                    tc.tile_pool(name="pso", bufs=2, space="PSUM")
                ),
            }

            # persistent inputs
            x2 = persist.tile([128, 65 * W], F32)  # two overlapping halves of x
            rhs0 = persist.tile([128, 80], F32)
            pwt = persist.tile([128, 2, 64], F32)
            gbias = persist.tile([128, R], F32)
            mbias = persist.tile([128, 12], F32)
            pwb = persist.tile([64, 1], F32)
            ident = persist.tile([128, 128], F32)
            make_identity(nc, ident)
            cb = {}
            for cname, cval in [
                ("eps", EPS),
                ("pi2", math.pi / 2),
                ("mh2n", -2.0 * MIN_HYP),
                ("mh2p", 2.0 * MIN_HYP),
                ("mln2", -LN2),
            ]:
                t = persist.tile([128, 1], F32, tag="cb_" + cname, name="cb_" + cname)
                nc.gpsimd.memset(t, cval)
                cb[cname] = t

            CH = 65 * W // 4  # 2080
            for c4 in range(4):
                nc.sync.dma_start(
                    out=x2[0:64, ds(c4 * CH, CH)], in_=x_d[:, ds(c4 * CH, CH)]
                )
                nc.sync.dma_start(
                    out=x2[64:128, ds(c4 * CH, CH)],
                    in_=x_d[:, ds(63 * W + c4 * CH, CH)],
                )
            nc.sync.dma_start(out=rhs0[0:64, :], in_=rhs0_d)
            nc.sync.dma_start(out=rhs0[64:128, :], in_=rhs0_d)
            nc.sync.dma_start(out=pwt, in_=pwt_d)
            nc.gpsimd.dma_start(
                out=gbias,
                in_=bass.AP(tensor=gb_d.tensor, offset=0, ap=[[0, 128], [1, R]]),
            )
            nc.gpsimd.dma_start(
                out=mbias,
                in_=bass.AP(tensor=mb_d.tensor, offset=0, ap=[[0, 128], [1, 12]]),
            )
            nc.sync.dma_start(out=pwb, in_=pb_d.rearrange("(c o) -> c o", o=1))

            for pair in range(H // PAIR_ROWS):
                gg = pools["gg"].tile([128, PAIR_ROWS, 16], F32, tag="gg")
                bts = []
                for b2 in range(2):
                    Bi = pair * 2 + b2
                    bts.append(_production(nc, pools, x2, rhs0, Bi, gg))
                G = _geometry(nc, pools, gg, gbias, mbias, cb)
                for b2 in range(2):
                    Bi = pair * 2 + b2
                    _mac_and_out(
                        nc, pools, Bi, bts[b2], G, b2 * BAND, ident, pwt, pwb, out_d
                    )
    nc.compile()
    return nc


_NC_CACHE = {}


def _get_nc():
    if "nc" not in _NC_CACHE:
        _NC_CACHE["nc"] = build_nc()
    return _NC_CACHE["nc"]


def prep_core_inputs(inputs, core):
    x = np.ascontiguousarray(inputs["x"][core].reshape(C, L), dtype=np.float32)
    value_w = inputs["value_w"].astype(np.float32)
    gate_w = inputs["gate_w"].astype(np.float32)
    geom_w = inputs["geom_w"].astype(np.float32)
    pw_w = inputs["pw_w"].astype(np.float32)
    rhs0 = np.concatenate([value_w.T, gate_w.T, geom_w.T], axis=1)  # [64, 80]
    pw_t2 = np.ascontiguousarray(
        pw_w.T.reshape(2, 128, 64).transpose(1, 0, 2)
    )  # [128, 2, 64]; pw_t2[p, h, o] = pw_w[o, h*128+p]
    return {
        "x": x,
        "rhs0": np.ascontiguousarray(rhs0),
        "pw_t2": pw_t2,
        "gate_b": inputs["gate_b"].astype(np.float32),
        "geom_b": inputs["geom_b"].astype(np.float32),
        "pw_b": inputs["pw_b"].astype(np.float32),
    }


def kernel(**inputs):
    from concourse.bass_utils import run_bass_kernel_spmd

    nc = _get_nc()
    inputs = {k: np.asarray(v) for k, v in inputs.items()}
    in_maps = [prep_core_inputs(inputs, i) for i in range(B)]
    res = run_bass_kernel_spmd(nc, in_maps, core_ids=list(range(B)))
    out = np.stack([r["out"].reshape(C, H, W) for r in res.results])
    return out.astype(np.float32)


if __name__ == "__main__":
    nc = build_nc()
    print("built ok")


# revision 13
# speedup vs baseline: 1.3404x; 1.3404x over previous
"""AZConv2d fused anisotropic conv kernel for Trainium2 (Bass/Tile).

Math (per pixel l, rule r):
  gate = gate_w@x + gate_b; mu = softmax_r(gate)
  v = value_w@x;  geom = geom_w@x + geom_b -> theta, raw_base, raw_hyper (4 each)
  base = softplus(rb)+1e-4; hyper = softplus(rh)+0.1
  iu2 = 1/(base*e^h)^2; is2 = 1/(base*e^-h)^2
  kern(dy,dx) = exp(-(a*dx^2 + b*dy^2 + c2*dx*dy)),
     a = ct^2*iu2 + st^2*is2, b = st^2*iu2+ct^2*is2, c2 = 2*ct*st*(iu2-is2)
  w = mu*kern / (sum_{r,s} mu*kern + 1e-6)
  agg[r,c] = sum_s w[r,s] * v[c, l+delta_s];  out = pw_w @ agg + pw_b

Since dx,dy in {-1,0,1}, kern takes 5 distinct values {1, A, B, AB*e^-c2, AB*e^c2}
(A=e^-a, B=e^-b), so with the mirrored-shift pair sums
  V0=v, V1=v(l-1)+v(l+1), V2=v(l-W)+v(l+W),
  S=V3+V4, T=V3-V4  (V3/V4 = main/anti diagonal pairs)
agg_r = G0*V0 + G1*V1 + G2*V2 + G3*S + G4*T with per-pixel scalars
  G0=em', G1=em'A, G2=em'B, G3=em'AB*cosh(c2), G4=-em'AB*sinh(c2),
  em' = e^{gate-max}/(sum_r e^{gate-max}*Sk_r + 1e-6*sum_r e^{gate-max}),
  Sk = 1+2A+2B+4AB*cosh(c2).
S/T come from horizontal pair sum/diff fields propagated via value conv:
  u = Wv@(x(l-1)+x(l+1)), wd = Wv@(x(l-1)-x(l+1));
  V1 = u, S = u(l-W)+u(l+W), T = wd(l-W)-wd(l+W).

Sharding: data-parallel over batch, 1 image per NeuronCore (8 cores).
Layout: pixel-major bands of 8 image rows; partitions = 128 pixels of a row.
"""

import math
import sys

for p in ("/opt/trn_rl_repo",):
    if p not in sys.path:
        sys.path.insert(0, p)

import ml_dtypes
import numpy as np

import concourse.bass as bass
import concourse.mybir as mybir
import concourse.tile as tile
from concourse import bacc
from concourse.bass import ds
from concourse.masks import make_identity

F32 = mybir.dt.float32
BF16 = mybir.dt.bfloat16
AF = mybir.ActivationFunctionType
ALU = mybir.AluOpType

B = 8
C = 64
H = 128
W = 128
L = H * W
R = 4
BAND = 8
NBANDS = H // BAND  # 16
PAIR_ROWS = 2 * BAND  # geometry granularity (16 rows)
EPS = 1e-4
MIN_HYP = 0.1
LN2 = math.log(2.0)


def _production(nc, pools, x2, rhs0, Bi, gg):
    """Produce pixel-major fields for band Bi (rows y0..y0+7) incl. 1-row halo.

    Returns (v_pm, u_pm, w_pm, v2, vs, vt): [128, rows, 64] tiles.
    v/u/w have 10 rows (j=0..9 -> image rows y0-1..y0+8, zero at image edge);
    v2/vs/vt have 8 rows (the band itself).
    Also copies this band's gate/geom (16 fields) into gg rows.
    """
    pband, pps, pact = pools["band"], pools["ps"], pools["act"]
    y0 = Bi * BAND
    jlo = 1 if Bi == 0 else 0
    jhi = 9 if Bi == NBANDS - 1 else 10
    half = 0 if Bi < NBANDS // 2 else 1
    p0 = 64 * half
    rowbase = 0 if half == 0 else 63  # x2 half-1 rows 0..64, half-2 rows 63..127

    def xrow(y):
        return x2[p0 : p0 + 64, ds((y - rowbase) * W, W)]

    n = jhi - jlo
    # c-major slice of x covering rows y0-1+jlo .. y0-1+jhi-1
    x2v = x2[p0 : p0 + 64, ds((y0 - 1 + jlo - rowbase) * W, n * W)].rearrange(
        "p (r w) -> p r w", w=W
    )

    # horizontal pair sum / diff of x (c-major), with zero-pad column fixes
    # (allocated full-height; only partitions p0..p0+64 are used, matching x2's half)
    x1f = pband.tile([128, 10, W], BF16, tag="x1")
    xdf = pband.tile([128, 10, W], BF16, tag="xd")
    x1 = x1f[p0 : p0 + 64]
    xd = xdf[p0 : p0 + 64]
    nc.vector.tensor_tensor(
        out=x1[:, jlo:jhi, 1 : W - 1],
        in0=x2v[:, :, 0 : W - 2],
        in1=x2v[:, :, 2:W],
        op=ALU.add,
    )
    nc.vector.tensor_copy(x1[:, jlo:jhi, 0:1], x2v[:, :, 1:2])
    nc.vector.tensor_copy(x1[:, jlo:jhi, W - 1 : W], x2v[:, :, W - 2 : W - 1])
    nc.vector.tensor_tensor(
        out=xd[:, jlo:jhi, 1 : W - 1],
        in0=x2v[:, :, 0 : W - 2],
        in1=x2v[:, :, 2:W],
        op=ALU.subtract,
    )
    nc.vector.tensor_scalar(
        out=xd[:, jlo:jhi, 0:1],
        in0=x2v[:, :, 1:2],
        scalar1=-1.0,
        scalar2=None,
        op0=ALU.mult,
    )
    nc.vector.tensor_copy(xd[:, jlo:jhi, W - 1 : W], x2v[:, :, W - 2 : W - 1])

    v_pm = pband.tile([128, 10, 64], BF16, tag="v")
    u_pm = pband.tile([128, 10, 64], BF16, tag="u")
    w_pm = pband.tile([128, 10, 64], BF16, tag="w")

    # value/gate/geom projections, pixel-major: out[pix, f] via lhsT = x row
    for g in range(2):  # groups of 5 rows for the 80-col projection
        ga, gb_ = max(g * 5, jlo), min(g * 5 + 5, jhi)
        if ga >= gb_:
            continue
        ps_v = pps.tile([128, 5, 80], F32, tag="psv")
        for j in range(ga, gb_):
            nc.tensor.matmul(
                out=ps_v[:, j - g * 5, :],
                lhsT=xrow(y0 - 1 + j),
                rhs=rhs0[p0 : p0 + 64, :],
                start=True,
                stop=True,
            )
        nc.scalar.activation(
            out=v_pm[:, ga:gb_, :], in_=ps_v[:, ga - g * 5 : gb_ - g * 5, 0:64],
            func=AF.Copy,
        )
        # gate/geom for the band's own rows (j=1..8) into the pair tile gg
        ia, ib = max(ga, 1), min(gb_, 9)
        if ia < ib:
            ggrow = (y0 % PAIR_ROWS) + ia - 1
            nc.scalar.activation(
                out=gg[:, ggrow : ggrow + (ib - ia), :],
                in_=ps_v[:, ia - g * 5 : ib - g * 5, 64:80],
                func=AF.Copy,
            )

    for g in range(3):  # groups of 4 rows for u/w projections
        ga, gb_ = max(g * 4, jlo), min(g * 4 + 4, jhi)
        if ga >= gb_:
            continue
        ps_uw = pps.tile([128, 4, 128], F32, tag="psuw")
        for j in range(ga, gb_):
            nc.tensor.matmul(
                out=ps_uw[:, j - g * 4, 0:64],
                lhsT=x1[:, j, :],
                rhs=rhs0[p0 : p0 + 64, 0:64],
                start=True,
                stop=True,
            )
            nc.tensor.matmul(
                out=ps_uw[:, j - g * 4, 64:128],
                lhsT=xd[:, j, :],
                rhs=rhs0[p0 : p0 + 64, 0:64],
                start=True,
                stop=True,
            )
        nc.scalar.activation(
            out=u_pm[:, ga:gb_, :], in_=ps_uw[:, ga - g * 4 : gb_ - g * 4, 0:64],
            func=AF.Copy,
        )
        nc.scalar.activation(
            out=w_pm[:, ga:gb_, :], in_=ps_uw[:, ga - g * 4 : gb_ - g * 4, 64:128],
            func=AF.Copy,
        )

    # zero-pad rows outside the image
    if jlo == 1:
        for t in (v_pm, u_pm, w_pm):
            nc.gpsimd.memset(t[:, 0:1, :], 0.0)
    if jhi == 9:
        for t in (v_pm, u_pm, w_pm):
            nc.gpsimd.memset(t[:, 9:10, :], 0.0)

    # vertical pair fields for the band
    v2 = pband.tile([128, BAND, 64], BF16, tag="v2")
    vs = pband.tile([128, BAND, 64], BF16, tag="vs")
    vt = pband.tile([128, BAND, 64], BF16, tag="vt")
    nc.vector.tensor_tensor(out=v2, in0=v_pm[:, 0:8, :], in1=v_pm[:, 2:10, :], op=ALU.add)
    nc.vector.tensor_tensor(out=vs, in0=u_pm[:, 0:8, :], in1=u_pm[:, 2:10, :], op=ALU.add)
    nc.vector.tensor_tensor(
        out=vt, in0=w_pm[:, 0:8, :], in1=w_pm[:, 2:10, :], op=ALU.subtract
    )
    return v_pm, u_pm, w_pm, v2, vs, vt


def _geometry(nc, pools, gg, gbias, mbias, cb):
    """Per-pixel aggregation coefficients G0..G4 [128, PAIR_ROWS, 4] from gg."""
    pg = pools["geo"]
    Y = PAIR_ROWS

    def gt(tag):
        return pg.tile([128, Y, R], F32, tag=tag, name=tag)

    gate = gg[:, :, 0:4]
    theta = gg[:, :, 4:8]

    # biases (broadcast along rows)
    nc.vector.tensor_tensor(
        out=gate, in0=gate, in1=gbias[:, None, :].broadcast_to([128, Y, 4]), op=ALU.add
    )
    nc.vector.tensor_tensor(
        out=gg[:, :, 4:16],
        in0=gg[:, :, 4:16],
        in1=mbias[:, None, :].broadcast_to([128, Y, 12]),
        op=ALU.add,
    )

    # trig first (sin table set), then everything exp/ln in one block to
    # minimize ACT table-set switches.
    ct, st = gt("ct"), gt("st")
    sh = gt("sh")
    nc.scalar.activation(out=st, in_=theta, func=AF.Sin)
    nc.scalar.activation(out=sh, in_=theta, func=AF.Sin, scale=0.5)
    sh2 = gt("sh2")
    nc.scalar.activation(out=sh2, in_=sh, func=AF.Square)
    nc.vector.tensor_scalar(
        out=ct, in0=sh2, scalar1=-2.0, scalar2=1.0, op0=ALU.mult, op1=ALU.add
    )

    mx = pg.tile([128, Y, 1], F32, tag="mx")
    nc.vector.tensor_reduce(out=mx, in_=gate, axis=mybir.AxisListType.X, op=ALU.max)
    gsh = gt("gsh")
    nc.vector.tensor_tensor(
        out=gsh, in0=gate, in1=mx.broadcast_to([128, Y, 4]), op=ALU.subtract
    )
    em = gt("em")
    nc.scalar.activation(out=em, in_=gsh, func=AF.Exp)

    e8 = pg.tile([128, Y, 8], F32, tag="e8")
    nc.scalar.activation(out=e8, in_=gg[:, :, 8:16], func=AF.Exp)
    sp8 = pg.tile([128, Y, 8], F32, tag="sp8")
    nc.scalar.activation(out=sp8, in_=e8, func=AF.Ln, bias=1.0)
    spb, sph = sp8[:, :, 0:4], sp8[:, :, 4:8]
    lb = gt("lb")
    nc.scalar.activation(out=lb, in_=spb, func=AF.Ln, bias=cb["eps"])

    tpl, tmi = gt("tpl"), gt("tmi")
    nc.vector.tensor_tensor(out=tpl, in0=sph, in1=lb, op=ALU.add)
    nc.vector.tensor_tensor(out=tmi, in0=sph, in1=lb, op=ALU.subtract)
    iu2, is2 = gt("iu2"), gt("is2")
    # 1/sig_u^2 = exp(-2(sph+0.1+lb)); 1/sig_s^2 = exp(2(sph+0.1-lb))
    nc.scalar.activation(out=iu2, in_=tpl, func=AF.Exp, scale=-2.0, bias=cb["mh2n"])
    nc.scalar.activation(out=is2, in_=tmi, func=AF.Exp, scale=2.0, bias=cb["mh2p"])

    ct2, st2 = gt("ct2"), gt("st2")
    nc.vector.tensor_tensor(out=ct2, in0=ct, in1=ct, op=ALU.mult)
    nc.vector.tensor_tensor(out=st2, in0=st, in1=st, op=ALU.mult)

    t1, t2, av = gt("t1"), gt("t2"), gt("av")
    nc.vector.tensor_tensor(out=t1, in0=ct2, in1=iu2, op=ALU.mult)
    nc.vector.tensor_tensor(out=t2, in0=st2, in1=is2, op=ALU.mult)
    nc.vector.tensor_tensor(out=av, in0=t1, in1=t2, op=ALU.add)
    ssum, bv = gt("ssum"), gt("bv")
    nc.vector.tensor_tensor(out=ssum, in0=iu2, in1=is2, op=ALU.add)
    nc.vector.tensor_tensor(out=bv, in0=ssum, in1=av, op=ALU.subtract)

    cs, dio, c2h = gt("cs"), gt("dio"), gt("c2h")
    nc.vector.tensor_tensor(out=cs, in0=ct, in1=st, op=ALU.mult)
    nc.vector.tensor_tensor(out=dio, in0=iu2, in1=is2, op=ALU.subtract)
    nc.vector.tensor_tensor(out=c2h, in0=cs, in1=dio, op=ALU.mult)

    Aa, Bb = gt("Aa"), gt("Bb")
    nc.scalar.activation(out=Aa, in_=av, func=AF.Exp, scale=-1.0)
    nc.scalar.activation(out=Bb, in_=bv, func=AF.Exp, scale=-1.0)
    # Corner kernels, overflow-safe: a+b = ssum >= |c2|, so compute
    # P/2 = exp(-(a+b+c2))/2 and Q/2 = exp(-(a+b-c2))/2 directly.
    sc2, sc2m = gt("sc2"), gt("sc2m")
    nc.vector.scalar_tensor_tensor(
        out=sc2, in0=c2h, scalar=2.0, in1=ssum, op0=ALU.mult, op1=ALU.add
    )
    nc.vector.scalar_tensor_tensor(
        out=sc2m, in0=c2h, scalar=-2.0, in1=ssum, op0=ALU.mult, op1=ALU.add
    )
    ph, qh = gt("ph"), gt("qh")
    nc.scalar.activation(out=ph, in_=sc2, func=AF.Exp, scale=-1.0, bias=cb["mln2"])
    nc.scalar.activation(out=qh, in_=sc2m, func=AF.Exp, scale=-1.0, bias=cb["mln2"])
    ppqh, pmqh = gt("ppqh"), gt("pmqh")
    nc.vector.tensor_tensor(out=ppqh, in0=ph, in1=qh, op=ALU.add)
    nc.vector.tensor_tensor(out=pmqh, in0=ph, in1=qh, op=ALU.subtract)

    # Sk = 1 + 2(A+B) + 4*(P+Q)/2
    apb, w1, sk = gt("apb"), gt("w1"), gt("sk")
    nc.vector.tensor_tensor(out=apb, in0=Aa, in1=Bb, op=ALU.add)
    nc.vector.scalar_tensor_tensor(
        out=w1, in0=ppqh, scalar=2.0, in1=apb, op0=ALU.mult, op1=ALU.add
    )
    nc.vector.tensor_scalar(
        out=sk, in0=w1, scalar1=2.0, scalar2=1.0, op0=ALU.mult, op1=ALU.add
    )

    ws = gt("ws")
    nc.vector.tensor_tensor(out=ws, in0=em, in1=sk, op=ALU.mult)
    Dp = pg.tile([128, Y, 1], F32, tag="Dp")
    Smu = pg.tile([128, Y, 1], F32, tag="Smu")
    nc.vector.tensor_reduce(out=Dp, in_=ws, axis=mybir.AxisListType.X, op=ALU.add)
    nc.vector.tensor_reduce(out=Smu, in_=em, axis=mybir.AxisListType.X, op=ALU.add)
    D2 = pg.tile([128, Y, 1], F32, tag="D2")
    nc.vector.scalar_tensor_tensor(
        out=D2, in0=Smu, scalar=1e-6, in1=Dp, op0=ALU.mult, op1=ALU.add
    )
    invD = pg.tile([128, Y, 1], F32, tag="invD")
    nc.vector.reciprocal(invD, D2)

    em2 = gt("em2")
    nc.vector.tensor_tensor(
        out=em2, in0=em, in1=invD.broadcast_to([128, Y, 4]), op=ALU.mult
    )
    G1, G2, G3, G4 = gt("G1"), gt("G2"), gt("G3"), gt("G4")
    nc.vector.tensor_tensor(out=G1, in0=em2, in1=Aa, op=ALU.mult)
    nc.vector.tensor_tensor(out=G2, in0=em2, in1=Bb, op=ALU.mult)
    nc.vector.tensor_tensor(out=G3, in0=em2, in1=ppqh, op=ALU.mult)
    nc.vector.tensor_tensor(out=G4, in0=em2, in1=pmqh, op=ALU.mult)
    return em2, G1, G2, G3, G4


def _mac_and_out(nc, pools, Bi, bt, G, goff, ident, pwt, pwb, out_d):
    """Weighted aggregation for band Bi, transpose to c-major, pointwise conv."""
    pband, pma, ptmp, ppsT, ppso, pout = (
        pools["band"],
        pools["mac"],
        pools["tmp"],
        pools["psT"],
        pools["pso"],
        pools["out"],
    )
    v_pm, u_pm, w_pm, v2, vs, vt = bt
    G0, G1, G2, G3, G4 = G
    y0 = Bi * BAND
    sh = [128, BAND, R, 64]

    def bg(t):  # coeff [128, 8, 4] -> bcast over channels
        return t[:, goff : goff + BAND, :, None].broadcast_to(sh)

    def bv(t):  # field [128, 8, 64] -> bcast over rules
        return t[:, :, None, :].broadcast_to(sh)

    agg = pma.tile(sh, BF16, tag="agg")
    tA = ptmp.tile(sh, BF16, tag="tA")
    tB = ptmp.tile(sh, BF16, tag="tB")
    nc.vector.tensor_tensor(out=tA, in0=bv(v_pm[:, 1:9, :]), in1=bg(G0), op=ALU.mult)
    nc.vector.tensor_tensor(out=tB, in0=bv(u_pm[:, 1:9, :]), in1=bg(G1), op=ALU.mult)
    nc.vector.tensor_tensor(out=agg, in0=tA, in1=tB, op=ALU.add)
    nc.vector.tensor_tensor(out=tA, in0=bv(v2), in1=bg(G2), op=ALU.mult)
    nc.vector.tensor_tensor(out=agg, in0=agg, in1=tA, op=ALU.add)
    nc.vector.tensor_tensor(out=tB, in0=bv(vs), in1=bg(G3), op=ALU.mult)
    nc.vector.tensor_tensor(out=agg, in0=agg, in1=tB, op=ALU.add)
    nc.vector.tensor_tensor(out=tA, in0=bv(vt), in1=bg(G4), op=ALU.mult)
    nc.vector.tensor_tensor(out=agg, in0=agg, in1=tA, op=ALU.add)

    # transpose [pixel, (r c)] -> [(r c), pixel] via PE, half (128 rc) at a time
    aggT = pma.tile([128, 2, BAND, 128], BF16, tag="aggT")
    for hg in range(2):
        for grp in range(2):
            psT = ppsT.tile([128, 4, 128], BF16, tag="psT")
            for yy in range(4):
                yr = grp * 4 + yy
                nc.tensor.transpose(
                    out=psT[:, yy, :],
                    in_=agg[:, yr, 2 * hg : 2 * hg + 2, :].rearrange(
                        "p a b -> p (a b)"
                    ),
                    identity=ident,
                )
            nc.scalar.activation(
                out=aggT[:, hg, grp * 4 : grp * 4 + 4, :], in_=psT, func=AF.Copy
            )

    # out = pw @ agg + pw_b (c-major), 512 pixels per psum chunk
    for ch in range(2):
        ps_o = ppso.tile([64, 512], F32, tag="pso")
        for hg in range(2):
            nc.tensor.matmul(
                out=ps_o,
                lhsT=pwt[:, hg, :],
                rhs=aggT[:, hg, ch * 4 : ch * 4 + 4, :].rearrange("p a b -> p (a b)"),
                start=(hg == 0),
                stop=(hg == 1),
            )
        osb = pout.tile([64, 512], F32, tag="osb")
        nc.scalar.activation(out=osb, in_=ps_o, func=AF.Identity, bias=pwb, scale=1.0)
        nc.sync.dma_start(out=out_d[:, ds(y0 * W + ch * 512, 512)], in_=osb)


def build_nc():
    nc = bacc.Bacc("TRN2")
    x_d = nc.dram_tensor("x", [C, L], F32, kind="ExternalInput").ap()
    rhs0_d = nc.dram_tensor("rhs0", [C, 80], BF16, kind="ExternalInput").ap()
    pwt_d = nc.dram_tensor("pw_t2", [128, 2, 64], BF16, kind="ExternalInput").ap()
    gb_d = nc.dram_tensor("gate_b", [R], F32, kind="ExternalInput").ap()
    mb_d = nc.dram_tensor("geom_b", [12], F32, kind="ExternalInput").ap()
    pb_d = nc.dram_tensor("pw_b", [C], F32, kind="ExternalInput").ap()
    out_d = nc.dram_tensor("out", [C, L], F32, kind="ExternalOutput").ap()

    with tile.TileContext(nc) as tc:
        import contextlib

        with contextlib.ExitStack() as ctx:
            persist = ctx.enter_context(tc.tile_pool(name="persist", bufs=1))
            pools = {
                "band": ctx.enter_context(tc.tile_pool(name="band", bufs=3)),
                "mac": ctx.enter_context(tc.tile_pool(name="mac", bufs=2)),
                "tmp": ctx.enter_context(tc.tile_pool(name="tmp", bufs=1)),
                "geo": ctx.enter_context(tc.tile_pool(name="geo", bufs=2)),
                "gg": ctx.enter_context(tc.tile_pool(name="gg", bufs=2)),
                "act": None,
                "out": ctx.enter_context(tc.tile_pool(name="out", bufs=2)),
                "ps": ctx.enter_context(tc.tile_pool(name="ps", bufs=2, space="PSUM")),
                "psT": ctx.enter_context(
                    tc.tile_pool(name="psT", bufs=2, space="PSUM")
                ),
                "pso": ctx.enter_context(
                    tc.tile_pool(name="pso", bufs=2, space="PSUM")
                ),
            }

            # persistent inputs
            x2 = persist.tile([128, 65 * W], BF16)  # two overlapping halves of x
            rhs0 = persist.tile([128, 80], BF16)
            pwt = persist.tile([128, 2, 64], BF16)
            gbias = persist.tile([128, R], F32)
            mbias = persist.tile([128, 12], F32)
            pwb = persist.tile([64, 1], F32)
            ident = persist.tile([128, 128], BF16)
            make_identity(nc, ident)
            cb = {}
            for cname, cval in [
                ("eps", EPS),
                ("pi2", math.pi / 2),
                ("mh2n", -2.0 * MIN_HYP),
                ("mh2p", 2.0 * MIN_HYP),
                ("mln2", -LN2),
            ]:
                t = persist.tile([128, 1], F32, tag="cb_" + cname, name="cb_" + cname)
                nc.gpsimd.memset(t, cval)
                cb[cname] = t

            CH = 65 * W // 4  # 2080
            for c4 in range(4):
                nc.gpsimd.dma_start(
                    out=x2[0:64, ds(c4 * CH, CH)], in_=x_d[:, ds(c4 * CH, CH)]
                )
                nc.gpsimd.dma_start(
                    out=x2[64:128, ds(c4 * CH, CH)],
                    in_=x_d[:, ds(63 * W + c4 * CH, CH)],
                )
            nc.sync.dma_start(out=rhs0[0:64, :], in_=rhs0_d)
            nc.sync.dma_start(out=rhs0[64:128, :], in_=rhs0_d)
            nc.sync.dma_start(out=pwt, in_=pwt_d)
            nc.gpsimd.dma_start(
                out=gbias,
                in_=bass.AP(tensor=gb_d.tensor, offset=0, ap=[[0, 128], [1, R]]),
            )
            nc.gpsimd.dma_start(
                out=mbias,
                in_=bass.AP(tensor=mb_d.tensor, offset=0, ap=[[0, 128], [1, 12]]),
            )
            nc.sync.dma_start(out=pwb, in_=pb_d.rearrange("(c o) -> c o", o=1))

            for pair in range(H // PAIR_ROWS):
                gg = pools["gg"].tile([128, PAIR_ROWS, 16], F32, tag="gg")
                bts = []
                for b2 in range(2):
                    Bi = pair * 2 + b2
                    bts.append(_production(nc, pools, x2, rhs0, Bi, gg))
                G = _geometry(nc, pools, gg, gbias, mbias, cb)
                for b2 in range(2):
                    Bi = pair * 2 + b2
                    _mac_and_out(
                        nc, pools, Bi, bts[b2], G, b2 * BAND, ident, pwt, pwb, out_d
                    )
    nc.compile()
    return nc


_NC_CACHE = {}


def _get_nc():
    if "nc" not in _NC_CACHE:
        _NC_CACHE["nc"] = build_nc()
    return _NC_CACHE["nc"]


def prep_core_inputs(inputs, core):
    x = np.ascontiguousarray(inputs["x"][core].reshape(C, L), dtype=np.float32)
    value_w = inputs["value_w"].astype(np.float32)
    gate_w = inputs["gate_w"].astype(np.float32)
    geom_w = inputs["geom_w"].astype(np.float32)
    pw_w = inputs["pw_w"].astype(np.float32)
    rhs0 = np.concatenate([value_w.T, gate_w.T, geom_w.T], axis=1)  # [64, 80]
    pw_t2 = np.ascontiguousarray(
        pw_w.T.reshape(2, 128, 64).transpose(1, 0, 2)
    )  # [128, 2, 64]; pw_t2[p, h, o] = pw_w[o, h*128+p]
    return {
        "x": x,
        "rhs0": np.ascontiguousarray(rhs0).astype(ml_dtypes.bfloat16),
        "pw_t2": pw_t2.astype(ml_dtypes.bfloat16),
        "gate_b": inputs["gate_b"].astype(np.float32),
        "geom_b": inputs["geom_b"].astype(np.float32),
        "pw_b": inputs["pw_b"].astype(np.float32),
    }


def kernel(**inputs):
    from concourse.bass_utils import run_bass_kernel_spmd

    nc = _get_nc()
    inputs = {k: np.asarray(v) for k, v in inputs.items()}
    in_maps = [prep_core_inputs(inputs, i) for i in range(B)]
    res = run_bass_kernel_spmd(nc, in_maps, core_ids=list(range(B)))
    out = np.stack([r["out"].reshape(C, H, W) for r in res.results])
    return out.astype(np.float32)


if __name__ == "__main__":
    nc = build_nc()
    print("built ok")
